# revision 64
# baseline (speedup 1.0000x reference)
"""Trainium2 Bass kernel for nn_Attention_15470472200471.

Sharding (8 cores): core c -> batch c//2, head-half c%2 (8 of 16 heads).
Host: layernorm (exact fp32), concat memories, transpose, bf16 cast.
Device (per core): K/V projections up front (column-split x DMA so the first
accumulation groups unblock early), Q projection interleaved per seq chunk,
dots^T = K @ Q^T with the two heads of a pair run concurrently on the PE
(row tiling via base-partition-derived tile_position), exp batched in
1024-wide activations from 2-bank PSUM groups with 6 of 17 groups offloaded
to the vector engine via a calibrated Schraudolph bit-trick exp
(i16 = round(A*dots + B) bitcast to bf16), AV with a ones-column appended
to V so the softmax denominator comes out of the PE for free, fast-approx
reciprocal + gpsimd broadcast + fused normalize-cast, out-projection.
Host: sum the two head-half partials per batch + bo.
"""

import numpy as np
import ml_dtypes

B, N, DIM = 4, 2048, 1024
HEADS, DHEAD = 16, 64
N_MEM = 64
NK = N + N_MEM            # 2112
HL = 8                    # local heads per core
IL = HL * DHEAD           # 512 local inner dim
P = 128
NCORES = 8
KC = DIM // P             # 8 contraction chunks over model dim
MCQ = IL // P             # 4 partition-chunks over local inner
SC = N // 512             # 4 seq chunks of 512
NKT = (NK + P - 1) // P   # 17 nk tiles (16 full + 1 of 64)
NCH = 2 * NKT             # 34 dots chunks per (s,pr): chunk c=(t,hh)
NGRP = (NCH + 2) // 3     # 12 exp groups of <=3 chunks

# Schraudolph bit-trick exp (bf16 out): bits = round(A*z + B), z = raw dots.
# Groups in DVE_GROUPS are computed on the vector engine instead of ScalarE.
A_EXP = 23.083120654223414     # 2^7/ln(2) * 0.125 (dots scale folded in)
B_EXP = 16249.0                # 127*128 - 7 (calibrated)
DVE_GROUPS = (2, 5, 8, 11, 14, 16)

_CACHE = {}


def _build_nc():
    import concourse.mybir as mybir
    import concourse.tile as tile
    from concourse import bacc

    f32 = mybir.dt.float32
    bf16 = mybir.dt.bfloat16
    i16 = mybir.dt.int16
    EXPF = mybir.ActivationFunctionType.Exp
    MULT = mybir.AluOpType.mult
    ADD = mybir.AluOpType.add

    nc = bacc.Bacc("TRN2", target_bir_lowering=False, debug=False)
    xkvT_d = nc.dram_tensor("xkvT", [DIM, NK], bf16, kind="ExternalInput")
    wq_d = nc.dram_tensor("wq", [DIM, IL], bf16, kind="ExternalInput")
    wk_d = nc.dram_tensor("wk", [DIM, IL], bf16, kind="ExternalInput")
    wv_d = nc.dram_tensor("wv", [DIM, IL], bf16, kind="ExternalInput")
    wo_d = nc.dram_tensor("wo", [IL, DIM], bf16, kind="ExternalInput")
    out_d = nc.dram_tensor("out", [N, DIM], f32, kind="ExternalOutput")

    k_chunks = [(0, 512), (512, 512), (1024, 512), (1536, 512), (2048, 64)]

    with tile.TileContext(nc) as tc:
        with (
            tc.tile_pool(name="big", bufs=1) as big,
            tc.tile_pool(name="ptp", bufs=2) as ptp,
            tc.tile_pool(name="otp", bufs=2) as otp,
            tc.tile_pool(name="small", bufs=2) as small,
            tc.tile_pool(name="outb", bufs=2) as outb,
            tc.tile_pool(name="qtp", bufs=2) as qtp,
            tc.tile_pool(name="psD", bufs=2, space="PSUM") as psD,
            tc.tile_pool(name="psO", bufs=2, space="PSUM") as psO,
            tc.tile_pool(name="psZ", bufs=2, space="PSUM") as psZ,
        ):
            # persistent weights / activations; DMA order: wk, x, wv, wq, wo
            # so the first K-proj matmuls unblock as early as possible
            wk_sb = big.tile([P, KC, IL], bf16, tag="wk")
            nc.sync.dma_start(wk_sb, wk_d[:, :].rearrange("(c p) n -> p c n", p=P))
            wq_sb = big.tile([P, KC, IL], bf16, tag="wq")
            wv_sb = big.tile([P, KC, IL], bf16, tag="wv")
            wo_sb = big.tile([P, MCQ, DIM], bf16, tag="wo")

            kt_sb = big.tile([P, MCQ, NK], bf16, tag="kt")         # K^T
            vaug_sb = big.tile([P, NKT, HL, 65], bf16, tag="va")   # [V_h | 1]
            nc.vector.memset(vaug_sb[:, :, :, 64:65], 1.0)


            # ---- K/V projections (x freed after attention deps resolve) ----
            with tc.tile_pool(name="xp", bufs=1) as xp:
                x_sb = xp.tile([P, KC, NK], bf16, tag="x")
                # column-split x DMA so the first K-proj groups (cols 0:1024)
                # unblock ~8us earlier than a full-tensor transfer
                for (o, w) in ((0, 1024), (1024, 1024), (2048, 64)):
                    for k in range(KC):
                        nc.sync.dma_start(
                            x_sb[:, k, o:o + w],
                            xkvT_d[k * P:(k + 1) * P, o:o + w])
                nc.sync.dma_start(
                    wv_sb, wv_d[:, :].rearrange("(c p) n -> p c n", p=P))
                nc.sync.dma_start(
                    wq_sb, wq_d[:, :].rearrange("(c p) n -> p c n", p=P))
                nc.sync.dma_start(
                    wo_sb, wo_d[:, :].rearrange("(c p) n -> p c n", p=P))

                # K^T = Wk.T @ xkv^T  [IL, NK]; 2 chunks per 2-bank psum tile
                # col-group outer so early x columns unblock all 4 m-chunks
                for grp in ((0, 1), (2, 3), (4,)):
                    for m in range(MCQ):
                        ps = psD.tile([P, 1024], f32, tag="d")
                        w_tot = 0
                        for j, ci in enumerate(grp):
                            o, w = k_chunks[ci]
                            for k in range(KC):
                                nc.tensor.matmul(
                                    ps[:, j * 512:j * 512 + w],
                                    wk_sb[:, k, m * P:(m + 1) * P],
                                    x_sb[:, k, o:o + w],
                                    start=(k == 0), stop=(k == KC - 1),
                                )
                            w_tot = j * 512 + w
                        o0 = k_chunks[grp[0]][0]
                        nc.scalar.copy(
                            out=kt_sb[:, m, o0:o0 + w_tot], in_=ps[:, :w_tot]
                        )


                # V = xkv @ Wv  [NK, IL] -> vaug[., t, h, 0:64]; 2 t per tile
                for tg in range(0, NKT, 2):
                    ts = list(range(tg, min(tg + 2, NKT)))
                    ps = psD.tile([P, 1024], f32, tag="d")
                    for j, t in enumerate(ts):
                        mt = P if t < 16 else 64
                        for k in range(KC):
                            nc.tensor.matmul(
                                ps[:mt, j * 512:(j + 1) * 512],
                                x_sb[:, k, t * P:t * P + mt],
                                wv_sb[:, k, 0:IL],
                                start=(k == 0), stop=(k == KC - 1),
                            )
                    full = [j for j, t in enumerate(ts) if t < 16]
                    nf = len(full)
                    if nf:
                        nc.vector.tensor_copy(
                            out=vaug_sb[:, ts[0]:ts[0] + nf, :, 0:64],
                            in_=ps[:, 0:nf * 512].rearrange(
                                "p (a h d) -> p a h d", a=nf, h=HL
                            ),
                        )
                    if ts[-1] == 16:
                        j = len(ts) - 1
                        nc.vector.tensor_copy(
                            out=vaug_sb[:64, 16, :, 0:64],
                            in_=ps[:64, j * 512:(j + 1) * 512].rearrange(
                                "p (h d) -> p h d", h=HL
                            ),
                        )

                # ---- attention; Q projection interleaved per s-chunk ----
                for s in range(SC):
                    qt_sb = qtp.tile([P, MCQ, 512], bf16, tag="qt")
                    for mg in ((0, 1), (2, 3)):
                        ps = psD.tile([P, 1024], f32, tag="d")
                        for j, m in enumerate(mg):
                            for k in range(KC):
                                nc.tensor.matmul(
                                    ps[:, j * 512:(j + 1) * 512],
                                    wq_sb[:, k, m * P:(m + 1) * P],
                                    x_sb[:, k, s * 512:(s + 1) * 512],
                                    start=(k == 0), stop=(k == KC - 1),
                                )
                        nm = len(mg)
                        nc.vector.tensor_copy(
                            out=qt_sb[:, mg[0]:mg[0] + nm, :],
                            in_=ps[:, 0:nm * 512].rearrange(
                                "p (a n) -> p a n", a=nm
                            ),
                        )
                    ot_sb = otp.tile([P, MCQ, 512], bf16, tag="ot")
                    # exp groups: chunk pairs; chunks 32,33 have 64 valid rows
                    # chunk c = 2t+hh
                    cgroups = [(i, i + 1) for i in range(0, NCH, 2)]
                    c2g = {}
                    for gi, cs in enumerate(cgroups):
                        for off, c in enumerate(cs):
                            c2g[c] = (gi, off, len(cs))
                    for pr in range(MCQ):  # head pair (2pr, 2pr+1)
                        pt = ptp.tile([P, NKT, 1024], bf16, tag="pt")
                        gtile = None
                        for t in range(NKT):
                            mt = P if t < 16 else 64
                            for hh in range(2):
                                c = 2 * t + hh
                                g, off, glen = c2g[c]
                                if off == 0:
                                    gtile = psD.tile([P, 1024], f32, tag="d")
                                nc.tensor.matmul(
                                    gtile[:mt, off * 512:off * 512 + 512],
                                    kt_sb[hh * 64:hh * 64 + 64, pr,
                                          t * P:t * P + mt],
                                    qt_sb[hh * 64:hh * 64 + 64, pr, :],
                                    start=True, stop=True,
                                )
                                if off == glen - 1:
                                    w = glen * 512
                                    c0 = cgroups[g][0]
                                    mg = 64 if c0 >= 32 else P
                                    dst = pt[:mg, g, 0:w]
                                    if g in DVE_GROUPS:
                                        nc.vector.tensor_scalar(
                                            dst.bitcast(i16),
                                            gtile[:mg, :w],
                                            A_EXP, B_EXP, MULT, ADD,
                                        )
                                    else:
                                        nc.scalar.activation(
                                            dst, gtile[:mg, :w], EXPF,
                                            scale=0.125,
                                        )
                        poab = []
                        for hh in range(2):
                            poab.append(psO.tile([P, 512], f32, tag="o",
                                                 name=f"po{hh}"))
                        for t in range(NKT):
                            mt = P if t < 16 else 64
                            for hh in range(2):
                                nc.tensor.matmul(
                                    poab[hh][0:65],
                                    vaug_sb[:mt, t, 2 * pr + hh, :],
                                    pt[:mt, t, hh * 512:hh * 512 + 512],
                                    start=(t == 0), stop=(t == NKT - 1),
                                )
                        for hh in range(2):
                            po = poab[hh]
                            den = small.tile([1, 512], f32, tag="den")
                            nc.vector.tensor_copy(
                                out=den, in_=po[64:65, 0:512]
                            )
                            inv = small.tile([1, 512], f32, tag="inv")
                            nc.vector.reciprocal_approx_fast(inv, den)
                            bc = small.tile([64, 512], f32, tag="bc")
                            nc.gpsimd.partition_broadcast(bc, inv)
                            nc.vector.tensor_mul(
                                out=ot_sb[hh * 64:hh * 64 + 64, pr, :],
                                in0=po[0:64],
                                in1=bc,
                            )
                    # out-projection for this seq chunk, direct PSUM->HBM
                    for st in range(4):
                        r0 = s * 512 + st * P
                        for d in range(2):
                            pz = psZ.tile([P, 512], f32, tag="z")
                            for ic in range(MCQ):
                                nc.tensor.matmul(
                                    pz,
                                    ot_sb[:, ic, st * P:(st + 1) * P],
                                    wo_sb[:, ic, d * 512:(d + 1) * 512],
                                    start=(ic == 0), stop=(ic == MCQ - 1),
                                )
                            ob = outb.tile([P, 512], f32, tag="ob")
                            nc.vector.tensor_copy(out=ob, in_=pz)
                            nc.sync.dma_start(
                                out_d[r0:r0 + P, d * 512:(d + 1) * 512], ob
                            )
    nc.compile()
    return nc


def kernel(**inputs):
    x = np.asarray(inputs["x"], np.float32)
    memories = np.asarray(inputs["memories"], np.float32)
    g = np.asarray(inputs["ln_gamma"], np.float32)
    beta = np.asarray(inputs["ln_beta"], np.float32)
    Wq = np.asarray(inputs["Wq"], np.float32)
    Wkv = np.asarray(inputs["Wkv"], np.float32)
    Wo = np.asarray(inputs["Wo"], np.float32)
    bo = np.asarray(inputs["bo"], np.float32)

    mu = x.mean(-1, keepdims=True)
    var = x.var(-1, keepdims=True)
    xn = (x - mu) / np.sqrt(var + 1e-5) * g + beta

    bf = ml_dtypes.bfloat16
    in_maps = []
    for c in range(NCORES):
        bb, half = c // 2, c % 2
        i0 = half * IL
        xkv = np.concatenate([xn[bb], memories], axis=0)  # [NK, DIM]
        in_maps.append({
            "xkvT": np.ascontiguousarray(xkv.T).astype(bf),
            "wq": np.ascontiguousarray(Wq[:, i0:i0 + IL]).astype(bf),
            "wk": np.ascontiguousarray(Wkv[:, i0:i0 + IL]).astype(bf),
            "wv": np.ascontiguousarray(Wkv[:, DIM + i0:DIM + i0 + IL]).astype(bf),
            "wo": np.ascontiguousarray(Wo[i0:i0 + IL, :]).astype(bf),
        })

    if "nc" not in _CACHE:
        _CACHE["nc"] = _build_nc()
    nc = _CACHE["nc"]

    import time as _time
    from concourse.bass_utils import run_bass_kernel_spmd
    t0 = _time.time()
    res = run_bass_kernel_spmd(nc, in_maps, list(range(NCORES)))
    t1 = _time.time()
    if getattr(res, "exec_time_ns", None):
        print(f"HW exec time: {res.exec_time_ns} ns")
        it = getattr(res, "instructions_and_trace", None)
        if it:
            print(f"trace path: {it[1]}")
    else:
        print(f"spmd call wall: {(t1 - t0) * 1e9:.0f} ns")

    out = np.empty((B, N, DIM), np.float32)
    for bb in range(B):
        out[bb] = (
            np.asarray(res.results[2 * bb]["out"], np.float32)
            + np.asarray(res.results[2 * bb + 1]["out"], np.float32)
            + bo
        )
    return out


# revision 65
# speedup vs baseline: 1.0060x; 1.0060x over previous
"""Trainium2 Bass kernel for nn_Attention_15470472200471.

Sharding (8 cores): core c -> batch c//2, head-half c%2 (8 of 16 heads).
Host: layernorm (exact fp32), concat memories, transpose, bf16 cast.
Device (per core): K/V projections up front (column-split x DMA so the first
accumulation groups unblock early), Q projection interleaved per seq chunk,
dots^T = K @ Q^T with the two heads of a pair run concurrently on the PE
(row tiling via base-partition-derived tile_position), exp batched in
1024-wide activations from 2-bank PSUM groups with 6 of 17 groups offloaded
to the vector engine via a calibrated Schraudolph bit-trick exp
(i16 = round(A*dots + B) bitcast to bf16), AV with a ones-column appended
to V so the softmax denominator comes out of the PE for free, fast-approx
reciprocal + gpsimd broadcast + fused normalize-cast, out-projection.
Host: sum the two head-half partials per batch + bo.
"""

import numpy as np
import ml_dtypes

B, N, DIM = 4, 2048, 1024
HEADS, DHEAD = 16, 64
N_MEM = 64
NK = N + N_MEM            # 2112
HL = 8                    # local heads per core
IL = HL * DHEAD           # 512 local inner dim
P = 128
NCORES = 8
KC = DIM // P             # 8 contraction chunks over model dim
MCQ = IL // P             # 4 partition-chunks over local inner
SC = N // 512             # 4 seq chunks of 512
NKT = (NK + P - 1) // P   # 17 nk tiles (16 full + 1 of 64)
NCH = 2 * NKT             # 34 dots chunks per (s,pr): chunk c=(t,hh)
NGRP = (NCH + 2) // 3     # 12 exp groups of <=3 chunks

# Schraudolph bit-trick exp (bf16 out): bits = round(A*z + B), z = raw dots.
# Groups in DVE_GROUPS are computed on the vector engine instead of ScalarE.
A_EXP = 23.083120654223414     # 2^7/ln(2) * 0.125 (dots scale folded in)
B_EXP = 16249.0                # 127*128 - 7 (calibrated)
DVE_GROUPS = (2, 5, 8, 11, 14, 16)

_CACHE = {}


def _build_nc():
    import concourse.mybir as mybir
    import concourse.tile as tile
    from concourse import bacc

    f32 = mybir.dt.float32
    bf16 = mybir.dt.bfloat16
    i16 = mybir.dt.int16
    EXPF = mybir.ActivationFunctionType.Exp
    MULT = mybir.AluOpType.mult
    ADD = mybir.AluOpType.add

    nc = bacc.Bacc("TRN2", target_bir_lowering=False, debug=False)
    xkvT_d = nc.dram_tensor("xkvT", [DIM, NK], bf16, kind="ExternalInput")
    wq_d = nc.dram_tensor("wq", [DIM, IL], bf16, kind="ExternalInput")
    wk_d = nc.dram_tensor("wk", [DIM, IL], bf16, kind="ExternalInput")
    wv_d = nc.dram_tensor("wv", [DIM, IL], bf16, kind="ExternalInput")
    wo_d = nc.dram_tensor("wo", [IL, DIM], bf16, kind="ExternalInput")
    out_d = nc.dram_tensor("out", [N, DIM], f32, kind="ExternalOutput")

    k_chunks = [(0, 512), (512, 512), (1024, 512), (1536, 512), (2048, 64)]

    with tile.TileContext(nc) as tc:
        with (
            tc.tile_pool(name="big", bufs=1) as big,
            tc.tile_pool(name="ptp", bufs=2) as ptp,
            tc.tile_pool(name="otp", bufs=2) as otp,
            tc.tile_pool(name="small", bufs=2) as small,
            tc.tile_pool(name="outb", bufs=2) as outb,
            tc.tile_pool(name="qtp", bufs=2) as qtp,
            tc.tile_pool(name="psD", bufs=2, space="PSUM") as psD,
            tc.tile_pool(name="psO", bufs=2, space="PSUM") as psO,
            tc.tile_pool(name="psZ", bufs=2, space="PSUM") as psZ,
        ):
            # persistent weights / activations; DMA order: wk, x, wv, wq, wo
            # so the first K-proj matmuls unblock as early as possible
            wk_sb = big.tile([P, KC, IL], bf16, tag="wk")
            nc.sync.dma_start(wk_sb, wk_d[:, :].rearrange("(c p) n -> p c n", p=P))
            wq_sb = big.tile([P, KC, IL], bf16, tag="wq")
            wv_sb = big.tile([P, KC, IL], bf16, tag="wv")
            wo_sb = big.tile([P, MCQ, DIM], bf16, tag="wo")

            kt_sb = big.tile([P, MCQ, NK], bf16, tag="kt")         # K^T
            vaug_sb = big.tile([P, NKT, HL, 65], bf16, tag="va")   # [V_h | 1]
            nc.vector.memset(vaug_sb[:, :, :, 64:65], 1.0)


            # ---- K/V projections (x freed after attention deps resolve) ----
            with tc.tile_pool(name="xp", bufs=1) as xp:
                x_sb = xp.tile([P, KC, NK], bf16, tag="x")
                # column-split x DMA so the first K-proj groups (cols 0:1024)
                # unblock ~8us earlier than a full-tensor transfer
                for (o, w) in ((0, 1024), (1024, 1024), (2048, 64)):
                    for k in range(KC):
                        nc.sync.dma_start(
                            x_sb[:, k, o:o + w],
                            xkvT_d[k * P:(k + 1) * P, o:o + w])
                nc.sync.dma_start(
                    wv_sb, wv_d[:, :].rearrange("(c p) n -> p c n", p=P))
                nc.sync.dma_start(
                    wq_sb, wq_d[:, :].rearrange("(c p) n -> p c n", p=P))
                nc.sync.dma_start(
                    wo_sb, wo_d[:, :].rearrange("(c p) n -> p c n", p=P))

                # K^T = Wk.T @ xkv^T  [IL, NK]; 2 chunks per 2-bank psum tile
                # col-group outer so early x columns unblock all 4 m-chunks
                for grp in ((0, 1), (2, 3), (4,)):
                    for m in range(MCQ):
                        ps = psD.tile([P, 1024], f32, tag="d")
                        w_tot = 0
                        for j, ci in enumerate(grp):
                            o, w = k_chunks[ci]
                            for k in range(KC):
                                nc.tensor.matmul(
                                    ps[:, j * 512:j * 512 + w],
                                    wk_sb[:, k, m * P:(m + 1) * P],
                                    x_sb[:, k, o:o + w],
                                    start=(k == 0), stop=(k == KC - 1),
                                )
                            w_tot = j * 512 + w
                        o0 = k_chunks[grp[0]][0]
                        nc.scalar.copy(
                            out=kt_sb[:, m, o0:o0 + w_tot], in_=ps[:, :w_tot]
                        )


                # V = xkv @ Wv  [NK, IL] -> vaug[., t, h, 0:64]; 2 t per tile
                for tg in range(0, NKT, 2):
                    ts = list(range(tg, min(tg + 2, NKT)))
                    ps = psD.tile([P, 1024], f32, tag="d")
                    for j, t in enumerate(ts):
                        mt = P if t < 16 else 64
                        for k in range(KC):
                            nc.tensor.matmul(
                                ps[:mt, j * 512:(j + 1) * 512],
                                x_sb[:, k, t * P:t * P + mt],
                                wv_sb[:, k, 0:IL],
                                start=(k == 0), stop=(k == KC - 1),
                            )
                    full = [j for j, t in enumerate(ts) if t < 16]
                    nf = len(full)
                    if nf:
                        nc.vector.tensor_copy(
                            out=vaug_sb[:, ts[0]:ts[0] + nf, :, 0:64],
                            in_=ps[:, 0:nf * 512].rearrange(
                                "p (a h d) -> p a h d", a=nf, h=HL
                            ),
                        )
                    if ts[-1] == 16:
                        j = len(ts) - 1
                        nc.vector.tensor_copy(
                            out=vaug_sb[:64, 16, :, 0:64],
                            in_=ps[:64, j * 512:(j + 1) * 512].rearrange(
                                "p (h d) -> p h d", h=HL
                            ),
                        )

                # ---- attention; Q projection interleaved per s-chunk ----
                for s in range(SC):
                    qt_sb = qtp.tile([P, MCQ, 512], bf16, tag="qt")
                    for mg in ((0, 1), (2, 3)):
                        ps = psD.tile([P, 1024], f32, tag="d")
                        for j, m in enumerate(mg):
                            for k in range(KC):
                                nc.tensor.matmul(
                                    ps[:, j * 512:(j + 1) * 512],
                                    wq_sb[:, k, m * P:(m + 1) * P],
                                    x_sb[:, k, s * 512:(s + 1) * 512],
                                    start=(k == 0), stop=(k == KC - 1),
                                )
                        nm = len(mg)
                        nc.vector.tensor_copy(
                            out=qt_sb[:, mg[0]:mg[0] + nm, :],
                            in_=ps[:, 0:nm * 512].rearrange(
                                "p (a n) -> p a n", a=nm
                            ),
                        )
                    ot_sb = otp.tile([P, MCQ, 512], bf16, tag="ot")
                    # exp groups: chunk pairs; chunks 32,33 have 64 valid rows
                    # chunk c = 2t+hh
                    cgroups = [(i, i + 1) for i in range(0, NCH, 2)]
                    c2g = {}
                    for gi, cs in enumerate(cgroups):
                        for off, c in enumerate(cs):
                            c2g[c] = (gi, off, len(cs))
                    for pr in range(MCQ):  # head pair (2pr, 2pr+1)
                        pt = ptp.tile([P, NKT, 1024], bf16, tag="pt")
                        gtile = None
                        for t in range(NKT):
                            mt = P if t < 16 else 64
                            for hh in range(2):
                                c = 2 * t + hh
                                g, off, glen = c2g[c]
                                if off == 0:
                                    gtile = psD.tile([P, 1024], f32, tag="d")
                                nc.tensor.matmul(
                                    gtile[:mt, off * 512:off * 512 + 512],
                                    kt_sb[hh * 64:hh * 64 + 64, pr,
                                          t * P:t * P + mt],
                                    qt_sb[hh * 64:hh * 64 + 64, pr, :],
                                    start=True, stop=True,
                                )
                                if off == glen - 1:
                                    w = glen * 512
                                    c0 = cgroups[g][0]
                                    mg = 64 if c0 >= 32 else P
                                    dst = pt[:mg, g, 0:w]
                                    if g in DVE_GROUPS:
                                        nc.vector.tensor_scalar(
                                            dst.bitcast(i16),
                                            gtile[:mg, :w],
                                            A_EXP, B_EXP, MULT, ADD,
                                        )
                                    else:
                                        nc.scalar.activation(
                                            dst, gtile[:mg, :w], EXPF,
                                            scale=0.125,
                                        )
                        for hh in range(2):
                            h = 2 * pr + hh
                            po = psO.tile([P, 512], f32, tag="o")
                            for t in range(NKT):
                                mt = P if t < 16 else 64
                                nc.tensor.matmul(
                                    po[0:65],
                                    vaug_sb[:mt, t, h, :],
                                    pt[:mt, t, hh * 512:hh * 512 + 512],
                                    start=(t == 0), stop=(t == NKT - 1),
                                )
                            den = small.tile([1, 512], f32, tag="den")
                            nc.vector.tensor_copy(
                                out=den, in_=po[64:65, 0:512]
                            )
                            inv = small.tile([1, 512], f32, tag="inv")
                            nc.vector.reciprocal_approx_fast(inv, den)
                            bc = small.tile([64, 512], f32, tag="bc")
                            nc.gpsimd.partition_broadcast(bc, inv)
                            nc.vector.tensor_mul(
                                out=ot_sb[hh * 64:hh * 64 + 64, pr, :],
                                in0=po[0:64],
                                in1=bc,
                            )
                    # out-projection for this seq chunk, direct PSUM->HBM
                    for st in range(4):
                        r0 = s * 512 + st * P
                        for d in range(2):
                            pz = psZ.tile([P, 512], f32, tag="z")
                            for ic in range(MCQ):
                                nc.tensor.matmul(
                                    pz,
                                    ot_sb[:, ic, st * P:(st + 1) * P],
                                    wo_sb[:, ic, d * 512:(d + 1) * 512],
                                    start=(ic == 0), stop=(ic == MCQ - 1),
                                )
                            ob = outb.tile([P, 512], f32, tag="ob")
                            nc.vector.tensor_copy(out=ob, in_=pz)
                            nc.sync.dma_start(
                                out_d[r0:r0 + P, d * 512:(d + 1) * 512], ob
                            )
    nc.compile()
    return nc


def kernel(**inputs):
    x = np.asarray(inputs["x"], np.float32)
    memories = np.asarray(inputs["memories"], np.float32)
    g = np.asarray(inputs["ln_gamma"], np.float32)
    beta = np.asarray(inputs["ln_beta"], np.float32)
    Wq = np.asarray(inputs["Wq"], np.float32)
    Wkv = np.asarray(inputs["Wkv"], np.float32)
    Wo = np.asarray(inputs["Wo"], np.float32)
    bo = np.asarray(inputs["bo"], np.float32)

    mu = x.mean(-1, keepdims=True)
    var = x.var(-1, keepdims=True)
    xn = (x - mu) / np.sqrt(var + 1e-5) * g + beta

    bf = ml_dtypes.bfloat16
    in_maps = []
    for c in range(NCORES):
        bb, half = c // 2, c % 2
        i0 = half * IL
        xkv = np.concatenate([xn[bb], memories], axis=0)  # [NK, DIM]
        in_maps.append({
            "xkvT": np.ascontiguousarray(xkv.T).astype(bf),
            "wq": np.ascontiguousarray(Wq[:, i0:i0 + IL]).astype(bf),
            "wk": np.ascontiguousarray(Wkv[:, i0:i0 + IL]).astype(bf),
            "wv": np.ascontiguousarray(Wkv[:, DIM + i0:DIM + i0 + IL]).astype(bf),
            "wo": np.ascontiguousarray(Wo[i0:i0 + IL, :]).astype(bf),
        })

    if "nc" not in _CACHE:
        _CACHE["nc"] = _build_nc()
    nc = _CACHE["nc"]

    import time as _time
    from concourse.bass_utils import run_bass_kernel_spmd
    t0 = _time.time()
    res = run_bass_kernel_spmd(nc, in_maps, list(range(NCORES)))
    t1 = _time.time()
    if getattr(res, "exec_time_ns", None):
        print(f"HW exec time: {res.exec_time_ns} ns")
        it = getattr(res, "instructions_and_trace", None)
        if it:
            print(f"trace path: {it[1]}")
    else:
        print(f"spmd call wall: {(t1 - t0) * 1e9:.0f} ns")

    out = np.empty((B, N, DIM), np.float32)
    for bb in range(B):
        out[bb] = (
            np.asarray(res.results[2 * bb]["out"], np.float32)
            + np.asarray(res.results[2 * bb + 1]["out"], np.float32)
            + bo
        )
    return out


# revision 66
# speedup vs baseline: 1.0151x; 1.0090x over previous
"""Trainium2 Bass kernel for nn_Attention_15470472200471.

Sharding (8 cores): core c -> batch c//2, head-half c%2 (8 of 16 heads).
Host: layernorm (exact fp32), concat memories, transpose, bf16 cast.
Device (per core): K/V projections up front (column-split x DMA so the first
accumulation groups unblock early), Q projection interleaved per seq chunk,
dots^T = K @ Q^T with the two heads of a pair run concurrently on the PE
(row tiling via base-partition-derived tile_position), exp batched in
1024-wide activations from 2-bank PSUM groups with 6 of 17 groups offloaded
to the vector engine via a calibrated Schraudolph bit-trick exp
(i16 = round(A*dots + B) bitcast to bf16), AV with a ones-column appended
to V so the softmax denominator comes out of the PE for free, fast-approx
reciprocal + gpsimd broadcast + fused normalize-cast, out-projection.
Host: sum the two head-half partials per batch + bo.
"""

import numpy as np
import ml_dtypes

B, N, DIM = 4, 2048, 1024
HEADS, DHEAD = 16, 64
N_MEM = 64
NK = N + N_MEM            # 2112
HL = 8                    # local heads per core
IL = HL * DHEAD           # 512 local inner dim
P = 128
NCORES = 8
KC = DIM // P             # 8 contraction chunks over model dim
MCQ = IL // P             # 4 partition-chunks over local inner
SC = N // 512             # 4 seq chunks of 512
NKT = (NK + P - 1) // P   # 17 nk tiles (16 full + 1 of 64)
NCH = 2 * NKT             # 34 dots chunks per (s,pr): chunk c=(t,hh)
NGRP = (NCH + 2) // 3     # 12 exp groups of <=3 chunks

# Schraudolph bit-trick exp (bf16 out): bits = round(A*z + B), z = raw dots.
# Groups in DVE_GROUPS are computed on the vector engine instead of ScalarE.
A_EXP = 23.083120654223414     # 2^7/ln(2) * 0.125 (dots scale folded in)
B_EXP = 16249.0                # 127*128 - 7 (calibrated)
DVE_GROUPS = (2, 5, 8, 11, 14, 16)

_CACHE = {}


def _build_nc():
    import concourse.mybir as mybir
    import concourse.tile as tile
    from concourse import bacc

    f32 = mybir.dt.float32
    bf16 = mybir.dt.bfloat16
    i16 = mybir.dt.int16
    EXPF = mybir.ActivationFunctionType.Exp
    MULT = mybir.AluOpType.mult
    ADD = mybir.AluOpType.add

    nc = bacc.Bacc("TRN2", target_bir_lowering=False, debug=False)
    xkvT_d = nc.dram_tensor("xkvT", [DIM, NK], bf16, kind="ExternalInput")
    wq_d = nc.dram_tensor("wq", [DIM, IL], bf16, kind="ExternalInput")
    wk_d = nc.dram_tensor("wk", [DIM, IL], bf16, kind="ExternalInput")
    wv_d = nc.dram_tensor("wv", [DIM, IL], bf16, kind="ExternalInput")
    wo_d = nc.dram_tensor("wo", [IL, DIM], bf16, kind="ExternalInput")
    out_d = nc.dram_tensor("out", [N, DIM], f32, kind="ExternalOutput")

    k_chunks = [(0, 512), (512, 512), (1024, 512), (1536, 512), (2048, 64)]

    with tile.TileContext(nc) as tc:
        with (
            tc.tile_pool(name="big", bufs=1) as big,
            tc.tile_pool(name="ptp", bufs=2) as ptp,
            tc.tile_pool(name="otp", bufs=2) as otp,
            tc.tile_pool(name="small", bufs=2) as small,
            tc.tile_pool(name="outb", bufs=2) as outb,
            tc.tile_pool(name="qtp", bufs=2) as qtp,
            tc.tile_pool(name="psD", bufs=2, space="PSUM") as psD,
            tc.tile_pool(name="psO", bufs=2, space="PSUM") as psO,
            tc.tile_pool(name="psZ", bufs=2, space="PSUM") as psZ,
        ):
            # persistent weights / activations; DMA order: wk, x, wv, wq, wo
            # so the first K-proj matmuls unblock as early as possible
            wk_sb = big.tile([P, KC, IL], bf16, tag="wk")
            nc.sync.dma_start(wk_sb, wk_d[:, :].rearrange("(c p) n -> p c n", p=P))
            wq_sb = big.tile([P, KC, IL], bf16, tag="wq")
            wv_sb = big.tile([P, KC, IL], bf16, tag="wv")
            wo_sb = big.tile([P, MCQ, DIM], bf16, tag="wo")

            kt_sb = big.tile([P, MCQ, NK], bf16, tag="kt")         # K^T
            vaug_sb = big.tile([P, NKT, HL, 65], bf16, tag="va")   # [V_h | 1]
            nc.vector.memset(vaug_sb[:, :, :, 64:65], 1.0)


            # ---- K/V projections (x freed after attention deps resolve) ----
            with tc.tile_pool(name="xp", bufs=1) as xp:
                x_sb = xp.tile([P, KC, NK], bf16, tag="x")
                # column-split x DMA so the first K-proj groups (cols 0:1024)
                # unblock ~8us earlier than a full-tensor transfer
                for (o, w) in ((0, 1024), (1024, 1024), (2048, 64)):
                    for k in range(KC):
                        nc.sync.dma_start(
                            x_sb[:, k, o:o + w],
                            xkvT_d[k * P:(k + 1) * P, o:o + w])
                nc.sync.dma_start(
                    wv_sb, wv_d[:, :].rearrange("(c p) n -> p c n", p=P))
                nc.sync.dma_start(
                    wq_sb, wq_d[:, :].rearrange("(c p) n -> p c n", p=P))
                nc.sync.dma_start(
                    wo_sb, wo_d[:, :].rearrange("(c p) n -> p c n", p=P))

                # K^T = Wk.T @ xkv^T  [IL, NK]; 2 chunks per 2-bank psum tile
                # col-group outer so early x columns unblock all 4 m-chunks
                for grp in ((0, 1), (2, 3)):
                    for m in range(MCQ):
                        ps = psD.tile([P, 1024], f32, tag="d")
                        w_tot = 0
                        for j, ci in enumerate(grp):
                            o, w = k_chunks[ci]
                            for k in range(KC):
                                nc.tensor.matmul(
                                    ps[:, j * 512:j * 512 + w],
                                    wk_sb[:, k, m * P:(m + 1) * P],
                                    x_sb[:, k, o:o + w],
                                    start=(k == 0), stop=(k == KC - 1),
                                )
                            w_tot = j * 512 + w
                        o0 = k_chunks[grp[0]][0]
                        nc.scalar.copy(
                            out=kt_sb[:, m, o0:o0 + w_tot], in_=ps[:, :w_tot]
                        )


                # V = xkv @ Wv  [NK, IL] -> vaug[., t, h, 0:64]; 2 t per tile
                for tg in range(0, NKT, 2):
                    ts = list(range(tg, min(tg + 2, NKT)))
                    ps = psD.tile([P, 1024], f32, tag="d")
                    for j, t in enumerate(ts):
                        mt = P if t < 16 else 64
                        for k in range(KC):
                            nc.tensor.matmul(
                                ps[:mt, j * 512:(j + 1) * 512],
                                x_sb[:, k, t * P:t * P + mt],
                                wv_sb[:, k, 0:IL],
                                start=(k == 0), stop=(k == KC - 1),
                            )
                    full = [j for j, t in enumerate(ts) if t < 16]
                    nf = len(full)
                    if nf:
                        nc.vector.tensor_copy(
                            out=vaug_sb[:, ts[0]:ts[0] + nf, :, 0:64],
                            in_=ps[:, 0:nf * 512].rearrange(
                                "p (a h d) -> p a h d", a=nf, h=HL
                            ),
                        )
                    if ts[-1] == 16:
                        j = len(ts) - 1
                        nc.vector.tensor_copy(
                            out=vaug_sb[:64, 16, :, 0:64],
                            in_=ps[:64, j * 512:(j + 1) * 512].rearrange(
                                "p (h d) -> p h d", h=HL
                            ),
                        )

                # K tail columns (need the last x DMA) issued after V
                # so the PE queue's early section never waits the x tail
                for m in range(MCQ):
                    ps = psD.tile([P, 1024], f32, tag="d", name=f"k4_{m}")
                    o, w = k_chunks[4]
                    for k in range(KC):
                        nc.tensor.matmul(
                            ps[:, 0:w],
                            wk_sb[:, k, m * P:(m + 1) * P],
                            x_sb[:, k, o:o + w],
                            start=(k == 0), stop=(k == KC - 1),
                        )
                    nc.scalar.copy(
                        out=kt_sb[:, m, o:o + w], in_=ps[:, :w]
                    )

                # ---- attention; Q projection interleaved per s-chunk ----
                for s in range(SC):
                    qt_sb = qtp.tile([P, MCQ, 512], bf16, tag="qt")
                    for mg in ((0, 1), (2, 3)):
                        ps = psD.tile([P, 1024], f32, tag="d")
                        for j, m in enumerate(mg):
                            for k in range(KC):
                                nc.tensor.matmul(
                                    ps[:, j * 512:(j + 1) * 512],
                                    wq_sb[:, k, m * P:(m + 1) * P],
                                    x_sb[:, k, s * 512:(s + 1) * 512],
                                    start=(k == 0), stop=(k == KC - 1),
                                )
                        nm = len(mg)
                        nc.vector.tensor_copy(
                            out=qt_sb[:, mg[0]:mg[0] + nm, :],
                            in_=ps[:, 0:nm * 512].rearrange(
                                "p (a n) -> p a n", a=nm
                            ),
                        )
                    ot_sb = otp.tile([P, MCQ, 512], bf16, tag="ot")
                    # exp groups: chunk pairs; chunks 32,33 have 64 valid rows
                    # chunk c = 2t+hh
                    cgroups = [(i, i + 1) for i in range(0, NCH, 2)]
                    c2g = {}
                    for gi, cs in enumerate(cgroups):
                        for off, c in enumerate(cs):
                            c2g[c] = (gi, off, len(cs))
                    for pr in range(MCQ):  # head pair (2pr, 2pr+1)
                        pt = ptp.tile([P, NKT, 1024], bf16, tag="pt")
                        gtile = None
                        for t in range(NKT):
                            mt = P if t < 16 else 64
                            for hh in range(2):
                                c = 2 * t + hh
                                g, off, glen = c2g[c]
                                if off == 0:
                                    gtile = psD.tile([P, 1024], f32, tag="d")
                                nc.tensor.matmul(
                                    gtile[:mt, off * 512:off * 512 + 512],
                                    kt_sb[hh * 64:hh * 64 + 64, pr,
                                          t * P:t * P + mt],
                                    qt_sb[hh * 64:hh * 64 + 64, pr, :],
                                    start=True, stop=True,
                                )
                                if off == glen - 1:
                                    w = glen * 512
                                    c0 = cgroups[g][0]
                                    mg = 64 if c0 >= 32 else P
                                    dst = pt[:mg, g, 0:w]
                                    if g in DVE_GROUPS:
                                        nc.vector.tensor_scalar(
                                            dst.bitcast(i16),
                                            gtile[:mg, :w],
                                            A_EXP, B_EXP, MULT, ADD,
                                        )
                                    else:
                                        nc.scalar.activation(
                                            dst, gtile[:mg, :w], EXPF,
                                            scale=0.125,
                                        )
                        for hh in range(2):
                            h = 2 * pr + hh
                            po = psO.tile([P, 512], f32, tag="o")
                            for t in range(NKT):
                                mt = P if t < 16 else 64
                                nc.tensor.matmul(
                                    po[0:65],
                                    vaug_sb[:mt, t, h, :],
                                    pt[:mt, t, hh * 512:hh * 512 + 512],
                                    start=(t == 0), stop=(t == NKT - 1),
                                )
                            den = small.tile([1, 512], f32, tag="den")
                            nc.vector.tensor_copy(
                                out=den, in_=po[64:65, 0:512]
                            )
                            inv = small.tile([1, 512], f32, tag="inv")
                            nc.vector.reciprocal_approx_fast(inv, den)
                            bc = small.tile([64, 512], f32, tag="bc")
                            nc.gpsimd.partition_broadcast(bc, inv)
                            nc.vector.tensor_mul(
                                out=ot_sb[hh * 64:hh * 64 + 64, pr, :],
                                in0=po[0:64],
                                in1=bc,
                            )
                    # out-projection for this seq chunk, direct PSUM->HBM
                    for st in range(4):
                        r0 = s * 512 + st * P
                        for d in range(2):
                            pz = psZ.tile([P, 512], f32, tag="z")
                            for ic in range(MCQ):
                                nc.tensor.matmul(
                                    pz,
                                    ot_sb[:, ic, st * P:(st + 1) * P],
                                    wo_sb[:, ic, d * 512:(d + 1) * 512],
                                    start=(ic == 0), stop=(ic == MCQ - 1),
                                )
                            ob = outb.tile([P, 512], f32, tag="ob")
                            nc.vector.tensor_copy(out=ob, in_=pz)
                            nc.sync.dma_start(
                                out_d[r0:r0 + P, d * 512:(d + 1) * 512], ob
                            )
    nc.compile()
    return nc


def kernel(**inputs):
    x = np.asarray(inputs["x"], np.float32)
    memories = np.asarray(inputs["memories"], np.float32)
    g = np.asarray(inputs["ln_gamma"], np.float32)
    beta = np.asarray(inputs["ln_beta"], np.float32)
    Wq = np.asarray(inputs["Wq"], np.float32)
    Wkv = np.asarray(inputs["Wkv"], np.float32)
    Wo = np.asarray(inputs["Wo"], np.float32)
    bo = np.asarray(inputs["bo"], np.float32)

    mu = x.mean(-1, keepdims=True)
    var = x.var(-1, keepdims=True)
    xn = (x - mu) / np.sqrt(var + 1e-5) * g + beta

    bf = ml_dtypes.bfloat16
    in_maps = []
    for c in range(NCORES):
        bb, half = c // 2, c % 2
        i0 = half * IL
        xkv = np.concatenate([xn[bb], memories], axis=0)  # [NK, DIM]
        in_maps.append({
            "xkvT": np.ascontiguousarray(xkv.T).astype(bf),
            "wq": np.ascontiguousarray(Wq[:, i0:i0 + IL]).astype(bf),
            "wk": np.ascontiguousarray(Wkv[:, i0:i0 + IL]).astype(bf),
            "wv": np.ascontiguousarray(Wkv[:, DIM + i0:DIM + i0 + IL]).astype(bf),
            "wo": np.ascontiguousarray(Wo[i0:i0 + IL, :]).astype(bf),
        })

    if "nc" not in _CACHE:
        _CACHE["nc"] = _build_nc()
    nc = _CACHE["nc"]

    import time as _time
    from concourse.bass_utils import run_bass_kernel_spmd
    t0 = _time.time()
    res = run_bass_kernel_spmd(nc, in_maps, list(range(NCORES)))
    t1 = _time.time()
    if getattr(res, "exec_time_ns", None):
        print(f"HW exec time: {res.exec_time_ns} ns")
        it = getattr(res, "instructions_and_trace", None)
        if it:
            print(f"trace path: {it[1]}")
    else:
        print(f"spmd call wall: {(t1 - t0) * 1e9:.0f} ns")

    out = np.empty((B, N, DIM), np.float32)
    for bb in range(B):
        out[bb] = (
            np.asarray(res.results[2 * bb]["out"], np.float32)
            + np.asarray(res.results[2 * bb + 1]["out"], np.float32)
            + bo
        )
    return out


# revision 67
# speedup vs baseline: 1.0759x; 1.0600x over previous
"""Trainium2 Bass kernel for nn_Attention_15470472200471.

Sharding (8 cores): core c -> batch c//2, head-half c%2 (8 of 16 heads).
Host: layernorm (exact fp32), concat memories, transpose, bf16 cast.
Device (per core): K/V projections up front (column-split x DMA so the first
accumulation groups unblock early), Q projection interleaved per seq chunk,
dots^T = K @ Q^T with the two heads of a pair run concurrently on the PE
(row tiling via base-partition-derived tile_position), exp batched in
1024-wide activations from 2-bank PSUM groups with 6 of 17 groups offloaded
to the vector engine via a calibrated Schraudolph bit-trick exp
(i16 = round(A*dots + B) bitcast to bf16), AV with a ones-column appended
to V so the softmax denominator comes out of the PE for free, fast-approx
reciprocal + gpsimd broadcast + fused normalize-cast, out-projection.
Host: sum the two head-half partials per batch + bo.
"""

import numpy as np
import ml_dtypes

B, N, DIM = 4, 2048, 1024
HEADS, DHEAD = 16, 64
N_MEM = 64
NK = N + N_MEM            # 2112
HL = 8                    # local heads per core
IL = HL * DHEAD           # 512 local inner dim
P = 128
NCORES = 8
KC = DIM // P             # 8 contraction chunks over model dim
MCQ = IL // P             # 4 partition-chunks over local inner
SC = N // 512             # 4 seq chunks of 512
NKT = (NK + P - 1) // P   # 17 nk tiles (16 full + 1 of 64)
NCH = 2 * NKT             # 34 dots chunks per (s,pr): chunk c=(t,hh)
NGRP = (NCH + 2) // 3     # 12 exp groups of <=3 chunks

# Schraudolph bit-trick exp (bf16 out): bits = round(A*z + B), z = raw dots.
# Groups in DVE_GROUPS are computed on the vector engine instead of ScalarE.
A_EXP = 23.083120654223414     # 2^7/ln(2) * 0.125 (dots scale folded in)
B_EXP = 16249.0                # 127*128 - 7 (calibrated)
DVE_GROUPS = (2, 5, 8, 11, 14, 16)

_CACHE = {}


def _build_nc():
    import concourse.mybir as mybir
    import concourse.tile as tile
    from concourse import bacc

    f32 = mybir.dt.float32
    bf16 = mybir.dt.bfloat16
    i16 = mybir.dt.int16
    EXPF = mybir.ActivationFunctionType.Exp
    MULT = mybir.AluOpType.mult
    ADD = mybir.AluOpType.add

    nc = bacc.Bacc("TRN2", target_bir_lowering=False, debug=False)
    xkvT_d = nc.dram_tensor("xkvT", [DIM, NK], bf16, kind="ExternalInput")
    wq_d = nc.dram_tensor("wq", [DIM, IL], bf16, kind="ExternalInput")
    wk_d = nc.dram_tensor("wk", [DIM, IL], bf16, kind="ExternalInput")
    wv_d = nc.dram_tensor("wv", [DIM, IL], bf16, kind="ExternalInput")
    wo_d = nc.dram_tensor("wo", [IL, DIM], bf16, kind="ExternalInput")
    out_d = nc.dram_tensor("out", [N, DIM], f32, kind="ExternalOutput")

    k_chunks = [(0, 512), (512, 512), (1024, 512), (1536, 512), (2048, 64)]

    with tile.TileContext(nc) as tc:
        with (
            tc.tile_pool(name="big", bufs=1) as big,
            tc.tile_pool(name="ptp", bufs=2) as ptp,
            tc.tile_pool(name="otp", bufs=2) as otp,
            tc.tile_pool(name="small", bufs=2) as small,
            tc.tile_pool(name="outb", bufs=2) as outb,
            tc.tile_pool(name="qtp", bufs=2) as qtp,
            tc.tile_pool(name="psD", bufs=2, space="PSUM") as psD,
            tc.tile_pool(name="psO", bufs=2, space="PSUM") as psO,
            tc.tile_pool(name="psZ", bufs=2, space="PSUM") as psZ,
        ):
            # persistent weights / activations; DMA order: wk, x, wv, wq, wo
            # so the first K-proj matmuls unblock as early as possible
            wk_sb = big.tile([P, KC, IL], bf16, tag="wk")
            nc.sync.dma_start(wk_sb, wk_d[:, :].rearrange("(c p) n -> p c n", p=P))
            wq_sb = big.tile([P, KC, IL], bf16, tag="wq")
            wv_sb = big.tile([P, KC, IL], bf16, tag="wv")
            wo_sb = big.tile([P, MCQ, DIM], bf16, tag="wo")

            kt_sb = big.tile([P, MCQ, NK], bf16, tag="kt")         # K^T
            vaug_sb = big.tile([P, NKT, HL, 65], bf16, tag="va")   # [V_h | 1]
            nc.vector.memset(vaug_sb[:, :, :, 64:65], 1.0)


            # ---- K/V projections (x freed after attention deps resolve) ----
            with tc.tile_pool(name="xp", bufs=1) as xp:
                x_sb = xp.tile([P, KC, NK], bf16, tag="x")
                # column-split x DMA so the first K-proj groups (cols 0:1024)
                # unblock ~8us earlier than a full-tensor transfer
                for (o, w) in ((0, 1024), (1024, 1024), (2048, 64)):
                    for k in range(KC):
                        nc.sync.dma_start(
                            x_sb[:, k, o:o + w],
                            xkvT_d[k * P:(k + 1) * P, o:o + w])
                nc.sync.dma_start(
                    wv_sb, wv_d[:, :].rearrange("(c p) n -> p c n", p=P))
                nc.sync.dma_start(
                    wq_sb, wq_d[:, :].rearrange("(c p) n -> p c n", p=P))
                nc.sync.dma_start(
                    wo_sb, wo_d[:, :].rearrange("(c p) n -> p c n", p=P))

                # K^T = Wk.T @ xkv^T  [IL, NK]; 2 chunks per 2-bank psum tile
                # col-group outer so early x columns unblock all 4 m-chunks
                for grp in ((0, 1), (2, 3)):
                    for m in range(MCQ):
                        ps = psD.tile([P, 1024], f32, tag="d")
                        w_tot = 0
                        for j, ci in enumerate(grp):
                            o, w = k_chunks[ci]
                            for k in range(KC):
                                nc.tensor.matmul(
                                    ps[:, j * 512:j * 512 + w],
                                    wk_sb[:, k, m * P:(m + 1) * P],
                                    x_sb[:, k, o:o + w],
                                    start=(k == 0), stop=(k == KC - 1),
                                )
                            w_tot = j * 512 + w
                        o0 = k_chunks[grp[0]][0]
                        nc.scalar.copy(
                            out=kt_sb[:, m, o0:o0 + w_tot], in_=ps[:, :w_tot]
                        )


                # V = xkv @ Wv  [NK, IL] -> vaug[., t, h, 0:64]; 2 t per tile
                for tg in range(0, NKT, 2):
                    ts = list(range(tg, min(tg + 2, NKT)))
                    ps = psD.tile([P, 1024], f32, tag="d")
                    for j, t in enumerate(ts):
                        mt = P if t < 16 else 64
                        for k in range(KC):
                            nc.tensor.matmul(
                                ps[:mt, j * 512:(j + 1) * 512],
                                x_sb[:, k, t * P:t * P + mt],
                                wv_sb[:, k, 0:IL],
                                start=(k == 0), stop=(k == KC - 1),
                            )
                    full = [j for j, t in enumerate(ts) if t < 16]
                    nf = len(full)
                    if nf:
                        nc.vector.tensor_copy(
                            out=vaug_sb[:, ts[0]:ts[0] + nf, :, 0:64],
                            in_=ps[:, 0:nf * 512].rearrange(
                                "p (a h d) -> p a h d", a=nf, h=HL
                            ),
                        )
                    if ts[-1] == 16:
                        j = len(ts) - 1
                        nc.vector.tensor_copy(
                            out=vaug_sb[:64, 16, :, 0:64],
                            in_=ps[:64, j * 512:(j + 1) * 512].rearrange(
                                "p (h d) -> p h d", h=HL
                            ),
                        )

                # K tail columns (need the last x DMA) issued after V
                # so the PE queue's early section never waits the x tail
                for m in range(MCQ):
                    ps = psD.tile([P, 1024], f32, tag="d", name=f"k4_{m}")
                    o, w = k_chunks[4]
                    for k in range(KC):
                        nc.tensor.matmul(
                            ps[:, 0:w],
                            wk_sb[:, k, m * P:(m + 1) * P],
                            x_sb[:, k, o:o + w],
                            start=(k == 0), stop=(k == KC - 1),
                        )
                    nc.scalar.copy(
                        out=kt_sb[:, m, o:o + w], in_=ps[:, :w]
                    )

                # ---- attention: software-pipelined issue order ----
                # AV+normalize for (s,pr) are issued AFTER dots/exp of the
                # next pr so PE-queue entries waiting on late exp groups
                # never block ready dots work (FIFO head-of-line).
                cgroups = [(i, i + 1) for i in range(0, NCH, 2)]
                c2g = {}
                for gi, cs in enumerate(cgroups):
                    for off, c in enumerate(cs):
                        c2g[c] = (gi, off, len(cs))

                def dots_exp(pt, qt_sb, pr):
                    gtile = None
                    for t in range(NKT):
                        mt = P if t < 16 else 64
                        for hh in range(2):
                            c = 2 * t + hh
                            g, off, glen = c2g[c]
                            if off == 0:
                                gtile = psD.tile([P, 1024], f32, tag="d",
                                                 name=f"g{pr}_{t}")
                            nc.tensor.matmul(
                                gtile[:mt, off * 512:off * 512 + 512],
                                kt_sb[hh * 64:hh * 64 + 64, pr,
                                      t * P:t * P + mt],
                                qt_sb[hh * 64:hh * 64 + 64, pr, :],
                                start=True, stop=True,
                            )
                            if off == glen - 1:
                                w = glen * 512
                                mg = 64 if cgroups[g][0] >= 32 else P
                                dst = pt[:mg, g, 0:w]
                                if g in DVE_GROUPS:
                                    nc.vector.tensor_scalar(
                                        dst.bitcast(i16), gtile[:mg, :w],
                                        A_EXP, B_EXP, MULT, ADD,
                                    )
                                else:
                                    nc.scalar.activation(
                                        dst, gtile[:mg, :w], EXPF,
                                        scale=0.125,
                                    )

                def av_norm(pt, pr, ot_sb):
                    for hh in range(2):
                        h = 2 * pr + hh
                        po = psO.tile([P, 512], f32, tag="o",
                                      name=f"po{pr}_{hh}")
                        for t in range(NKT):
                            mt = P if t < 16 else 64
                            nc.tensor.matmul(
                                po[0:65],
                                vaug_sb[:mt, t, h, :],
                                pt[:mt, t, hh * 512:hh * 512 + 512],
                                start=(t == 0), stop=(t == NKT - 1),
                            )
                        den = small.tile([1, 512], f32, tag="den",
                                         name=f"dn{pr}{hh}")
                        nc.vector.tensor_copy(out=den, in_=po[64:65, 0:512])
                        inv = small.tile([1, 512], f32, tag="inv",
                                         name=f"iv{pr}{hh}")
                        nc.vector.reciprocal_approx_fast(inv, den)
                        bc = small.tile([64, 512], f32, tag="bc",
                                        name=f"bc{pr}{hh}")
                        nc.gpsimd.partition_broadcast(bc, inv)
                        nc.vector.tensor_mul(
                            out=ot_sb[hh * 64:hh * 64 + 64, pr, :],
                            in0=po[0:64],
                            in1=bc,
                        )

                def outproj(s, ot_sb):
                    for st in range(4):
                        r0 = s * 512 + st * P
                        for dd in range(2):
                            pz = psZ.tile([P, 512], f32, tag="z",
                                          name=f"pz{s}{st}{dd}")
                            for ic in range(MCQ):
                                nc.tensor.matmul(
                                    pz,
                                    ot_sb[:, ic, st * P:(st + 1) * P],
                                    wo_sb[:, ic, dd * 512:(dd + 1) * 512],
                                    start=(ic == 0), stop=(ic == MCQ - 1),
                                )
                            ob = outb.tile([P, 512], f32, tag="ob",
                                           name=f"ob{s}{st}{dd}")
                            nc.vector.tensor_copy(out=ob, in_=pz)
                            nc.sync.dma_start(
                                out_d[r0:r0 + P,
                                      dd * 512:(dd + 1) * 512], ob
                            )

                pend = None       # (pt, pr, ot) awaiting AV+normalize
                odone = None      # (s, ot) awaiting outproj
                for s in range(SC):
                    qt_sb = qtp.tile([P, MCQ, 512], bf16, tag="qt",
                                     name=f"qt{s}")
                    for mg in ((0, 1), (2, 3)):
                        ps = psD.tile([P, 1024], f32, tag="d",
                                      name=f"qp{s}{mg[0]}")
                        for j, m in enumerate(mg):
                            for k in range(KC):
                                nc.tensor.matmul(
                                    ps[:, j * 512:(j + 1) * 512],
                                    wq_sb[:, k, m * P:(m + 1) * P],
                                    x_sb[:, k, s * 512:(s + 1) * 512],
                                    start=(k == 0), stop=(k == KC - 1),
                                )
                        nc.vector.tensor_copy(
                            out=qt_sb[:, mg[0]:mg[0] + 2, :],
                            in_=ps.rearrange("p (a n) -> p a n", a=2),
                        )
                    ot_sb = otp.tile([P, MCQ, 512], bf16, tag="ot",
                                     name=f"ot{s}")
                    for pr in range(MCQ):
                        pt = ptp.tile([P, NKT, 1024], bf16, tag="pt",
                                      name=f"pt{s}{pr}")
                        dots_exp(pt, qt_sb, pr)
                        if pend is not None:
                            av_norm(*pend)
                        pend = (pt, pr, ot_sb)
                        if pr == 0 and odone is not None:
                            outproj(*odone)
                            odone = None
                    odone = (s, ot_sb)
                av_norm(*pend)
                outproj(*odone)
    nc.compile()
    return nc


def kernel(**inputs):
    x = np.asarray(inputs["x"], np.float32)
    memories = np.asarray(inputs["memories"], np.float32)
    g = np.asarray(inputs["ln_gamma"], np.float32)
    beta = np.asarray(inputs["ln_beta"], np.float32)
    Wq = np.asarray(inputs["Wq"], np.float32)
    Wkv = np.asarray(inputs["Wkv"], np.float32)
    Wo = np.asarray(inputs["Wo"], np.float32)
    bo = np.asarray(inputs["bo"], np.float32)

    mu = x.mean(-1, keepdims=True)
    var = x.var(-1, keepdims=True)
    xn = (x - mu) / np.sqrt(var + 1e-5) * g + beta

    bf = ml_dtypes.bfloat16
    in_maps = []
    for c in range(NCORES):
        bb, half = c // 2, c % 2
        i0 = half * IL
        xkv = np.concatenate([xn[bb], memories], axis=0)  # [NK, DIM]
        in_maps.append({
            "xkvT": np.ascontiguousarray(xkv.T).astype(bf),
            "wq": np.ascontiguousarray(Wq[:, i0:i0 + IL]).astype(bf),
            "wk": np.ascontiguousarray(Wkv[:, i0:i0 + IL]).astype(bf),
            "wv": np.ascontiguousarray(Wkv[:, DIM + i0:DIM + i0 + IL]).astype(bf),
            "wo": np.ascontiguousarray(Wo[i0:i0 + IL, :]).astype(bf),
        })

    if "nc" not in _CACHE:
        _CACHE["nc"] = _build_nc()
    nc = _CACHE["nc"]

    import time as _time
    from concourse.bass_utils import run_bass_kernel_spmd
    t0 = _time.time()
    res = run_bass_kernel_spmd(nc, in_maps, list(range(NCORES)))
    t1 = _time.time()
    if getattr(res, "exec_time_ns", None):
        print(f"HW exec time: {res.exec_time_ns} ns")
        it = getattr(res, "instructions_and_trace", None)
        if it:
            print(f"trace path: {it[1]}")
    else:
        print(f"spmd call wall: {(t1 - t0) * 1e9:.0f} ns")

    out = np.empty((B, N, DIM), np.float32)
    for bb in range(B):
        out[bb] = (
            np.asarray(res.results[2 * bb]["out"], np.float32)
            + np.asarray(res.results[2 * bb + 1]["out"], np.float32)
            + bo
        )
    return out
